# revision 14
# baseline (speedup 1.0000x reference)
"""AttentivePooling Trainium2 kernel.

Reference math (per batch b):
    q  = query @ Wq.T + bq                      [H]
    qc = q @ Wc                                 [H]   (folds the c-projection)
    cb = q . bc                                 scalar
    att[n,s]  = context[n,s,:] . qc + cb
    w = softmax(att * mask, axis=s)
    result[n,h]       = sum_s w[n,s] * context[n,s,h]
    token_result[s,h] = sum_n w[n,s] * context[n,s,h]

Sharding: 8 cores = 4 batches x 2 halves of N (100 segments each).
q/qc/cb (128-dim projections, ~0.003% of FLOPs) are computed on host as
part of input prep; token_result partials of core pairs are summed on host.

On-device layout: [s_in=128 partitions, (n, c, h) free], fp16 via
cast-during-DMA (SWDGE).  att = DVE multiply + halving tree over h;
softmax uses PE ones-matmuls for the cross-partition sums; the weighted
sums run on PE (w-column stationary matmuls for result rows; identity
stationary matmuls PSUM-accumulating token across all segments).
"""

from contextlib import ExitStack

import numpy as np

import concourse.bass as bass
import concourse.mybir as mybir
import concourse.tile as tile
from concourse import bacc

B, N, S, H = 4, 200, 512, 128
C = S // 128  # 4 chunks of 128 along s
F16 = mybir.dt.float16
F32 = mybir.dt.float32

# (block_segments, att_sub_segments, tmp_chunks_on_ACT) per pipeline block
DEFAULT_PLAN = [(12, 6, 16), (20, 10, 32), (20, 10, 32), (20, 10, 32), (20, 10, 24), (8, 4, 4)]


def build_nc(nseg=100, plan=None, num_devices=8):
    """Build the single-core bass program (SPMD: same NEFF on all cores)."""
    if plan is None:
        plan = DEFAULT_PLAN
    assert sum(p[0] for p in plan) == nseg
    assert all(b % 4 == 0 and b % s == 0 for b, s, _ in plan)
    max_sub = max(p[1] for p in plan)
    max_blk = max(p[0] for p in plan)

    nc = bacc.Bacc(
        "TRN2",
        target_bir_lowering=False,
        debug=False,
        enable_asserts=False,
        num_devices=num_devices,
    )

    x = nc.dram_tensor("x", [nseg, S, H], F32, kind="ExternalInput").ap()
    mask = nc.dram_tensor("mask", [nseg, S], F32, kind="ExternalInput").ap()
    qc1 = nc.dram_tensor("qc1", [128, H], F16, kind="ExternalInput").ap()
    cb = nc.dram_tensor("cb", [128, 1], F32, kind="ExternalInput").ap()
    ident16 = nc.dram_tensor("ident16", [128, 128], F16, kind="ExternalInput").ap()
    ident32 = nc.dram_tensor("ident32", [128, 128], F32, kind="ExternalInput").ap()
    res = nc.dram_tensor("res", [nseg, H], F32, kind="ExternalOutput").ap()
    tok = nc.dram_tensor("tok", [S, H], F32, kind="ExternalOutput").ap()

    # DRAM view: [s_in(partition), n, c, h]
    x_r = x.rearrange("n (c p) h -> p n c h", c=C, p=128)

    with nc.allow_low_precision("fp16 attention pipeline (validated vs fp32 ref)"):
        with tile.TileContext(nc) as tc, ExitStack() as ctx:
            # ---------------- pools ----------------
            const_pool = ctx.enter_context(tc.tile_pool(name="const", bufs=1))
            x_pool = ctx.enter_context(tc.tile_pool(name="x16", bufs=4))
            prod_pool = ctx.enter_context(tc.tile_pool(name="prod", bufs=2))
            tmp_pool = ctx.enter_context(tc.tile_pool(name="tmp", bufs=2))
            att_pool = ctx.enter_context(tc.tile_pool(name="att", bufs=2))
            small_pool = ctx.enter_context(tc.tile_pool(name="small", bufs=2))
            tok_psum = ctx.enter_context(tc.tile_pool(name="tokps", bufs=1, space="PSUM"))
            res_psum = ctx.enter_context(tc.tile_pool(name="resps", bufs=3, space="PSUM"))
            z_psum = ctx.enter_context(tc.tile_pool(name="zps", bufs=3, space="PSUM"))

            # ---------------- setup ----------------
            # first context sub-load goes first so DVE can start ASAP
            blk0, sub0, _ = plan[0]
            x16_0 = x_pool.tile([128, max_blk, C, H], F16, tag="x16")
            nc.gpsimd.dma_start(x16_0[:, 0:sub0], x_r[:, 0:sub0])

            qc_sb = const_pool.tile([128, max_sub * C * H], F16)
            nc.sync.dma_start(qc_sb[:, 0:H], qc1[:])
            # log-doubling replication (runs during the first x load)
            rep = H
            while rep < max_sub * C * H:
                n = min(rep, max_sub * C * H - rep)
                nc.vector.tensor_copy(qc_sb[:, rep : rep + n], qc_sb[:, 0:n])
                rep += n
            cb_sb = const_pool.tile([128, 1], F32)
            nc.sync.dma_start(cb_sb[:], cb[:])
            id16_sb = const_pool.tile([128, 128], F16)
            nc.sync.dma_start(id16_sb[:], ident16[:])

            ones32 = const_pool.tile([128, 1], F32)
            nc.vector.memset(ones32[:], 1.0)
            ones32r = const_pool.tile([1, 128], F32)
            nc.vector.memset(ones32r[:], 1.0)

            # mask -> mask_T [s_in, (n, c)] via PE transposes
            mask_T = const_pool.tile([128, nseg, C], F32)
            with tc.tile_pool(name="msetup", bufs=1) as mpool:
                id32_sb = mpool.tile([128, 128], F32)
                nc.sync.dma_start(id32_sb[:], ident32[:])
                m_nat = mpool.tile([nseg, S], F32)
                nc.sync.dma_start(m_nat[:], mask[:])
                for c in range(C):
                    mps = z_psum.tile([128, nseg], F32, tag="zps")
                    nc.tensor.transpose(
                        mps[:], m_nat[:, bass.ts(c, 128)], id32_sb[:nseg, :nseg]
                    )
                    nc.scalar.copy(mask_T[:, :, c], mps[:])

            tok_sb = const_pool.tile([128, C * H], F32)

            # ---------------- main pipeline ----------------
            tps = tok_psum.tile([128, C * H], F32, tag="tps")
            base = 0
            for j, (blk, sub, n_act) in enumerate(plan):
                nsub = blk // sub
                last = j == len(plan) - 1

                x16 = (
                    x16_0
                    if j == 0
                    else x_pool.tile([128, max_blk, C, H], F16, tag="x16")
                )
                for k in range(nsub):
                    if j == 0 and k == 0:
                        continue
                    n0 = base + k * sub
                    nc.gpsimd.dma_start(
                        x16[:, k * sub : (k + 1) * sub], x_r[:, n0 : n0 + sub]
                    )
                x16f = x16.rearrange("p n c h -> p (n c h)")

                # --- attention scores ---
                att32 = att_pool.tile([128, max_blk * C], F32, tag="att32")
                for k in range(nsub):
                    prod = prod_pool.tile([128, max_sub * C * H], F16, tag="prod")
                    pk = prod[:, 0 : sub * C * H]
                    nc.vector.tensor_mul(
                        pk,
                        x16f[:, k * sub * C * H : (k + 1) * sub * C * H],
                        qc_sb[:, 0 : sub * C * H],
                    )
                    pv = pk.rearrange("p (g h) -> p g h", h=H)
                    w_ = H
                    while w_ > 8:
                        w_ //= 2
                        nc.vector.tensor_add(
                            pv[:, :, 0:w_], pv[:, :, 0:w_], pv[:, :, w_ : 2 * w_]
                        )
                    nc.vector.tensor_reduce(
                        att32[:, k * sub * C : (k + 1) * sub * C],
                        pv[:, :, 0:8],
                        axis=mybir.AxisListType.X,
                        op=mybir.AluOpType.add,
                    )

                # --- softmax (fp32; value range makes max-subtraction unnecessary) ---
                e32 = att_pool.tile([128, max_blk * C], F32, tag="e32")
                nc.vector.scalar_tensor_tensor(
                    e32[:, 0 : blk * C],
                    att32[:, 0 : blk * C],
                    cb_sb[:],
                    mask_T[:, base : base + blk].rearrange("p n c -> p (n c)"),
                    op0=mybir.AluOpType.add,
                    op1=mybir.AluOpType.mult,
                )
                nc.scalar.activation(
                    e32[:, 0 : blk * C],
                    e32[:, 0 : blk * C],
                    mybir.ActivationFunctionType.Exp,
                )

                en = small_pool.tile([128, max_blk], F32, tag="en")
                nc.vector.tensor_reduce(
                    en[:, 0:blk],
                    e32[:, 0 : blk * C].rearrange("p (n c) -> p n c", c=C),
                    axis=mybir.AxisListType.X,
                    op=mybir.AluOpType.add,
                )
                zps = z_psum.tile([1, max_blk], F32, tag="zps")
                nc.tensor.matmul(zps[:, 0:blk], ones32[:], en[:, 0:blk])
                rz = small_pool.tile([1, max_blk], F32, tag="rz")
                nc.vector.reciprocal(rz[:, 0:blk], zps[:, 0:blk])
                rzb = z_psum.tile([128, max_blk], F32, tag="zps")
                nc.tensor.matmul(rzb[:, 0:blk], ones32r[:], rz[:, 0:blk])

                w32 = att_pool.tile([128, max_blk * C], F32, tag="w32")
                rzb_b = (
                    rzb[:, 0:blk]
                    .rearrange("p (n one) -> p n one", one=1)
                    .broadcast_to([128, blk, C])
                )
                nc.vector.tensor_mul(
                    w32[:, 0 : blk * C].rearrange("p (n c) -> p n c", c=C),
                    e32[:, 0 : blk * C].rearrange("p (n c) -> p n c", c=C),
                    rzb_b,
                )
                w16 = att_pool.tile([128, max_blk * C], F16, tag="w16")
                nc.vector.tensor_copy(w16[:, 0 : blk * C], w32[:, 0 : blk * C])

                # --- weighted sums (per 4-segment group) ---
                res_sb = small_pool.tile([1, max_blk * H], F32, tag="res_sb")
                tmp = tmp_pool.tile([128, max_blk, C, H], F16, tag="tmp")
                tmpf = tmp.rearrange("p n c h -> p (n c h)")
                for g in range(blk // 4):
                    rps = res_psum.tile([1, 4 * H], F32, tag="rps")
                    for sl in range(4):
                        nl = g * 4 + sl
                        for c in range(C):
                            idx = nl * C + c
                            if idx >= blk * C - n_act:
                                nc.scalar.mul(
                                    tmp[:, nl, c], x16[:, nl, c], w32[:, idx : idx + 1]
                                )
                            else:
                                nc.vector.tensor_scalar_mul(
                                    tmp[:, nl, c], x16[:, nl, c], w32[:, idx : idx + 1]
                                )
                        for c in range(C):
                            nc.tensor.matmul(
                                rps[:, sl * H : (sl + 1) * H],
                                w16[:, nl * C + c : nl * C + c + 1],
                                x16[:, nl, c],
                                start=(c == 0),
                                stop=(c == C - 1),
                                skip_group_check=True,
                            )
                        nc.tensor.matmul(
                            tps[:],
                            id16_sb[:],
                            tmpf[:, nl * C * H : (nl + 1) * C * H],
                            start=(j == 0 and nl == 0),
                            stop=(last and nl == blk - 1),
                            skip_group_check=True,
                        )
                    nc.scalar.copy(res_sb[:, g * 4 * H : (g + 1) * 4 * H], rps[:])
                nc.sync.dma_start(res[base : base + blk], res_sb[:, 0 : blk * H])
                base += blk

            # ---------------- epilogue ----------------
            nc.scalar.copy(tok_sb[:], tps[:])
            tok_out = tok.rearrange("(c p) h -> p c h", p=128)
            nc.sync.dma_start(tok_out[:], tok_sb.rearrange("p (c h) -> p c h", h=H))

    nc.compile()
    return nc


_NC_CACHE = {}


def _get_nc(key=None):
    if key not in _NC_CACHE:
        _NC_CACHE[key] = build_nc()
    return _NC_CACHE[key]


def make_in_maps(query, context, context_mask, Wq, bq, Wc, bc, nseg=100):
    query = np.asarray(query, np.float32)
    context = np.asarray(context, np.float32)
    context_mask = np.asarray(context_mask, np.float32)
    Wq = np.asarray(Wq, np.float32)
    bq = np.asarray(bq, np.float32)
    Wc = np.asarray(Wc, np.float32)
    bc = np.asarray(bc, np.float32)

    q = query @ Wq.T + bq  # [B, H]
    qc = q @ Wc  # [B, H]
    cbv = (q * bc).sum(-1)  # [B]

    ident16 = np.eye(128, dtype=np.float16)
    ident32 = np.eye(128, dtype=np.float32)

    in_maps = []
    for core in range(8):
        b, half = core // 2, core % 2
        sl = slice(half * nseg, (half + 1) * nseg)
        in_maps.append(
            {
                "x": np.ascontiguousarray(context[b, sl]),
                "mask": np.ascontiguousarray(context_mask[b, sl]),
                "qc1": np.tile(qc[b].astype(np.float16), (128, 1)),
                "cb": np.full((128, 1), cbv[b], np.float32),
                "ident16": ident16,
                "ident32": ident32,
            }
        )
    return in_maps


def gather_outputs(results):
    result = np.stack(
        [
            np.concatenate([results[2 * b]["res"], results[2 * b + 1]["res"]], axis=0)
            for b in range(B)
        ]
    )
    token = np.stack(
        [results[2 * b]["tok"] + results[2 * b + 1]["tok"] for b in range(B)]
    )
    return result.astype(np.float32), token.astype(np.float32)


def kernel(query, context, context_mask, Wq, bq, Wc, bc):
    from concourse.bass_utils import run_bass_kernel_spmd

    nc = _get_nc()
    in_maps = make_in_maps(query, context, context_mask, Wq, bq, Wc, bc)
    out = run_bass_kernel_spmd(nc, in_maps, core_ids=list(range(8)))
    return gather_outputs(out.results)
